# revision 69
# baseline (speedup 1.0000x reference)
"""Fused Conv3d + per-batch global stats kernel for Trainium2 (8 NeuronCores).

Problem: x [16,64,32,32,32] f32, conv_weight [128,64,3,3,3], conv_bias [128].
  y = conv3d(x, w, VALID) + b        -> [16,128,30,30,30]
  out[n] = mean_n / sqrt(var_n + eps) over (C,D,H,W)   -> [16] f32

Strategy:
  - Data parallel: batch 16 -> 8 cores x 2 batches, weights replicated.
  - Conv as 27 tap-matmuls contracting Cin=64, accumulated in PSUM.
    y never materialized in HBM: only per-channel sum / sum-of-squares
    (bias folded analytically at the end).
  - K=64 matmuls packed 2x via PE row tiling: taps 0..13 on array rows
    0-63 (tile_position (0,0), PSUM bank A), taps 14..26 on rows 64-127
    (tile_position (64,0), PSUM bank B). x is duplicated on SBUF
    partitions 64-127 so each half streams independently. Banks are
    combined during the stats reduction (row tiles must not share a
    PSUM bank).
  - BF16 matmul datapath for the conv stream (on-device cast fp32 ->
    bf16; PSUM accumulation stays fp32; rel err ~1.7e-3 vs 2e-2 gate).
    bf16 streams ~10% faster than float32r on this HW (f32r measured
    ~0.475ns/col sustained vs bf16 ~0.43) -- worth 33us end-to-end.
    The terminal ones-matmul cross-partition reduce stays f32r.
  - Per-od output rows in 2 PSUM chunks (510+390); garbage columns
    excluded via strided APs in the matmul rhs so PSUM is densely
    packed with valid positions.
  - Stats per chunk: ScalarE stages psA to SBUF, DVE adds psB and
    reduces into a per-chunk scratch column, ScalarE Square accumulates
    ym^2 into a second scratch column. Scratch columns are reduced once
    per batch -- no serial S/S2 accumulator chain of tiny ops.
  - Head: plane DMAs issued before the weight DMAs; weights arrive in
    4 quarter-tiles with pipelined f32r rounding; the PE is prewarmed
    on memset junk from ~7us so the HAM clock gate is released before
    real matmuls start.
  - Tail: cross-partition reduce via SBUF->SBUF DMA flatten + strided
    DVE reduce, short scalar_tensor_tensor finalize chain, one combined
    out DMA for both batches.
  (Note: DVE tensor_tensor_reduce, GpSimd partition_all_reduce, and a
  terminal fp32 ones-matmul reduce were all tried and CRASH the exec
  unit on this HW/runtime -- keep to baseline-exercised instructions.
  Also measured: GpSimd dma_start is software-DGE and costs ~2us of
  steady-state when used for mid-kernel plane loads; a stride-0
  broadcast-source dma_start doubles total runtime; Scalar-engine
  dma_starts serialize on a single ~100GB/s ring.)
"""
import os
os.environ.setdefault("NEURON_RT_RESET_CORES", "1")

import numpy as np
from contextlib import ExitStack

import concourse.bass as bass
import concourse.tile as tile
from concourse import bacc, bass_isa, mybir
from concourse.bass_utils import run_bass_kernel_spmd

N_CORES = 8
CIN, COUT, KK = 64, 128, 3
D = H = W = 32
PL = H * W                      # 1024 linear positions per D-plane
OD = OH = OW = 30
NPOS = OD * OH * OW             # 27000 valid positions per (n, c)
NTOT = COUT * NPOS
EPS = 1e-5
NB = 2                          # batches per core
TAPS = [(kd, kh, kw) for kd in range(KK) for kh in range(KK) for kw in range(KK)]
# Two tap splits, alternated per chunk so each row tile averages 13.5
# matmuls (27 taps can't split evenly into 2x row tiles).
SPLITS = [(TAPS[:14], TAPS[14:]),   # 14 on T0, 13 on T8
          (TAPS[:13], TAPS[13:])]   # 13 on T0, 14 on T8
# (row0, n_valid_cols, n_oh_rows): matmul rhs streams only the 30 valid
# ow columns per oh row via a 3D strided AP, so PSUM is densely packed
# with valid positions (900 streamed cols/plane instead of 958).
CHUNKS = [(0, 510, 17), (17, 390, 13)]
NCHUNK = OD * len(CHUNKS)       # stats scratch columns per batch
QT = 7 * 128                    # weight-quarter width (7 taps)

F32 = mybir.dt.float32
F32R = mybir.dt.float32r
BF16 = mybir.dt.bfloat16
ADD = mybir.AluOpType.add
MULT = mybir.AluOpType.mult
SUB = mybir.AluOpType.subtract


def _emit(nc):
    x_ap = nc.dram_tensor("x", [NB, CIN, D * PL], F32, kind="ExternalInput").ap()
    wq_ap = nc.dram_tensor("wq", [128, 2 * 14 * 128], F32, kind="ExternalInput").ap()
    b_ap = nc.dram_tensor("bias", [128, 1], F32, kind="ExternalInput").ap()
    out_ap = nc.dram_tensor("out", [1, NB], F32, kind="ExternalOutput").ap()

    with tile.TileContext(nc) as tc, ExitStack() as ctx:
        wpool = ctx.enter_context(tc.tile_pool(name="w", bufs=1))
        cpool = ctx.enter_context(tc.tile_pool(name="const", bufs=1))
        stpool = ctx.enter_context(tc.tile_pool(name="stage", bufs=4))
        xgpool = ctx.enter_context(tc.tile_pool(name="xg", bufs=7))
        pspool = ctx.enter_context(tc.tile_pool(name="ps", bufs=8, space="PSUM"))
        aspool = ctx.enter_context(tc.tile_pool(name="as", bufs=4))
        ympool = ctx.enter_context(tc.tile_pool(name="ym", bufs=4))
        y2pool = ctx.enter_context(tc.tile_pool(name="y2", bufs=4))
        accpool = ctx.enter_context(tc.tile_pool(name="acc", bufs=2))
        finpool = ctx.enter_context(tc.tile_pool(name="fin", bufs=1))

        # --- PE prewarm on junk: HAM runs the PE cold (half clock) until
        # ~3.4us of sustained matmul activity. Burn the ramp on memset
        # data so it starts right after the framework preamble, during
        # the DMA head, instead of after the first real operands land.
        junks = cpool.tile([128, 512], F32, tag="junks")
        junkx = cpool.tile([128, 512], BF16, tag="junkx")
        junkw = cpool.tile([128, 128], BF16, tag="junkw")
        nc.vector.memset(junks[:, :], 0.0)
        nc.vector.tensor_copy(junkx[:, :], junks[:, :])
        nc.vector.tensor_copy(junkw[:, :], junks[:, 0:128])
        # ~3.4us of cold-rate junk. Longer junk streams (10/16 slots) and
        # alternate DMA issue orders all measured 2-4us WORSE end-to-end
        # despite firing the HAM un-throttle earlier -- 8 slots ending
        # ~2us before the first real matmul was the best measured
        # configuration (the sub-3.4us handoff gap cannot re-throttle).
        NJUNK = 8
        pwA = pspool.tile([128, 512], F32, tag="ps")
        pwB = pspool.tile([128, 512], F32, tag="ps")
        for i in range(NJUNK):
            nc.tensor.matmul(pwA[:, 0:512], junkw[0:64, :], junkx[0:64, :],
                             start=(i == 0), stop=(i == NJUNK - 1),
                             tile_position=(0, 0))
            nc.tensor.matmul(pwB[:, 0:512], junkw[64:128, :], junkx[64:128, :],
                             start=(i == 0), stop=(i == NJUNK - 1),
                             tile_position=(64, 0))

        # --- one-time loads. Planes 0-2 are issued before the weights so
        # the first-plane cast isn't queued behind the 1.8MB weight DMA.
        stage = {}

        def load_plane_dma(b, p):
            # two plain dma_starts; a single stride-0 broadcast-source
            # dma_start for both halves measured 2x SLOWER end-to-end
            # (739us) -- broadcast reads serialize in the DMA engine.
            # Head planes are cast inline, upper half on the
            # idle-at-head ScalarE (emitted before the wqr copies in
            # its queue), so the 6 casts don't serialize on DVE.
            st = stpool.tile([128, PL], F32, tag="st")
            src = x_ap[b][:, p * PL:(p + 1) * PL]
            nc.sync.dma_start(st[0:64, :], src)
            nc.sync.dma_start(st[64:128, :], src)
            t = xgpool.tile([128, PL + 32], BF16, tag="xg")
            nc.vector.tensor_copy(t[0:64, 0:PL], st[0:64, :])
            nc.scalar.copy(t[64:128, 0:PL], st[64:128, :])
            stage[(b, p)] = t

        # weights in 4 quarter-tiles (7 taps each) with pipelined f32r
        # rounding on the otherwise-idle ScalarE; the first chunk's
        # matmuls only wait for the quarters they read. Plane and weight
        # DMA issues are interleaved so neither gates the other.
        wqr = [wpool.tile([128, QT], BF16, tag=f"wqr{q}", name=f"wqr{q}")
               for q in range(4)]

        def load_wq(q):
            wq = wpool.tile([128, QT], F32, tag=f"wq{q}")
            nc.sync.dma_start(wq[:, :], wq_ap[:, q * QT:(q + 1) * QT])
            if q == 0:
                # q0 gates the first matmul slot: split its rounding
                # copy across DVE and ScalarE to halve the latency
                nc.vector.tensor_copy(wqr[0][:, 0:QT // 2], wq[:, 0:QT // 2])
                nc.scalar.copy(wqr[0][:, QT // 2:], wq[:, QT // 2:])
            else:
                nc.scalar.copy(wqr[q][:, :], wq[:, :])    # round to f32r

        for p in range(3):
            load_plane_dma(0, p)
        for q in range(4):
            load_wq(q)

        def wslice(split, half, i):
            # weight AP for tap i of row-half `half` in split `split`
            q, col = divmod(split * 14 + i, 7)
            rows = slice(0, 64) if half == 0 else slice(64, 128)
            return wqr[q][rows, col * 128:(col + 1) * 128]

        bias_t = cpool.tile([128, 1], F32, tag="bias")
        nc.sync.dma_start(bias_t[:, :], b_ap[:, :])
        negeps_t = cpool.tile([1, 1], F32, tag="negeps")
        nc.vector.memset(negeps_t[:, :], -EPS * float(NTOT) * float(NTOT))
        # ones-column matrix for the terminal PE cross-partition reduce:
        # lhsT [64,128] with col 0 = 1 makes a row-tiled f32r matmul sum
        # its 64 rhs partitions into out row 0. Same dtype + tile mode as
        # the conv stream, so no PE mode switch (an fp32 (128,32)-mode
        # reduce here crashes the exec unit).
        onesf = cpool.tile([128, 128], F32, tag="onesf")
        onesm = cpool.tile([128, 128], F32R, tag="onesm")
        nc.vector.memset(onesf[:, :], 0.0)
        nc.vector.memset(onesf[:, 0:1], 1.0)
        nc.vector.tensor_copy(onesm[:, :], onesf[:, :])

        # batch-independent bias reductions: cb = sum_c(b), sum_c(b^2).
        # Cross-partition reduce: flatten [128,2] -> [1,256] via
        # SBUF-to-SBUF DMA, then DVE-reduce the 128-long stride.
        cbin = cpool.tile([128, 2], F32, tag="cbin")
        nc.vector.tensor_copy(cbin[:, 0:1], bias_t[:, 0:1])
        nc.vector.tensor_mul(cbin[:, 1:2], bias_t[:, 0:1], bias_t[:, 0:1])
        cbcat = cpool.tile([1, 256], F32, tag="cbcat")
        nc.sync.dma_start(cbcat[0:1, 0:256], cbin[:, 0:2])
        cb = cpool.tile([1, 2], F32, tag="cb")
        nc.vector.tensor_reduce(
            cb[0:1, 0:2],
            cbcat[0:1, 0:256].rearrange("p (a b) -> p b a", b=2),
            axis=mybir.AxisListType.X, op=ADD)

        resb = finpool.tile([1, NB], F32, tag="resb")

        for b in range(NB):
            sscr = accpool.tile([128, NCHUNK], F32, tag="sscr")
            s2scr = accpool.tile([128, NCHUNK], F32, tag="s2scr")

            xp = {}

            def load_plane(p, b=b):
                if p in xp or p >= D:
                    return
                if (b, p) in stage:
                    xp[p] = stage.pop((b, p))   # pre-cast at the head
                    return
                else:
                    st = stpool.tile([128, PL], F32, tag="st")
                    src = x_ap[b][:, p * PL:(p + 1) * PL]
                    nc.sync.dma_start(st[0:64, :], src)
                    nc.sync.dma_start(st[64:128, :], src)
                # +32 col slack: the strided rhs slice of the last oh row
                # spans past PL (its b>=30 tail is never addressed)
                t = xgpool.tile([128, PL + 32], BF16, tag="xg")
                # f32r rounding on DVE. (Moving the upper-half cast to
                # ScalarE to parallelize the head measured +0.4us: the
                # 60 extra mid-kernel ACT copies interfere with the
                # per-chunk PSUM staging copies.)
                nc.vector.tensor_copy(t[0:64, 0:PL], st[0:64, :])
                nc.vector.tensor_copy(t[64:128, 0:PL], st[64:128, :])
                xp[p] = t

            for p in range(3):
                load_plane(p)

            chunk_idx = 0
            for od in range(OD):
                load_plane(od + 3)
                load_plane(od + 4)
                for g in [g for g in xp if g < od]:
                    del xp[g]

                for (r0, NC, NROW) in CHUNKS:
                    split = chunk_idx % 2
                    ta, tb = SPLITS[split]
                    ci = chunk_idx
                    chunk_idx += 1
                    psA = pspool.tile([128, 512], F32, tag="ps")
                    psB = pspool.tile([128, 512], F32, tag="ps")
                    for i in range(max(len(ta), len(tb))):
                        if i < len(ta):
                            kd, kh, kw = ta[i]
                            off = kh * W + kw + r0 * W
                            nc.tensor.matmul(
                                psA[:, 0:NC],
                                wslice(split, 0, i),
                                xp[od + kd][0:64, off:off + NROW * W].rearrange(
                                    "p (a b) -> p a b", b=W)[:, :, 0:OW],
                                start=(i == 0), stop=(i == len(ta) - 1),
                                tile_position=(0, 0))
                        if i < len(tb):
                            kd, kh, kw = tb[i]
                            off = kh * W + kw + r0 * W
                            nc.tensor.matmul(
                                psB[:, 0:NC],
                                wslice(split, 1, i),
                                xp[od + kd][64:128, off:off + NROW * W].rearrange(
                                    "p (a b) -> p a b", b=W)[:, :, 0:OW],
                                start=(i == 0), stop=(i == len(tb) - 1),
                                tile_position=(64, 0))

                    # stats: ym = psA + psB, row-sum into the chunk's
                    # scratch column; Square accumulates the row-sum of
                    # ym^2 into the second scratch. (The DVE can read
                    # only one PSUM operand per instruction, so ScalarE
                    # stages psA into SBUF first. Splitting the terminal
                    # chunk into pipelined column halves measured +1.2us
                    # -- per-op fixed costs exceed the overlap gain.)
                    aS = aspool.tile([128, 512], BF16, tag="aS")
                    nc.scalar.copy(aS[:, 0:NC], psA[:, 0:NC])
                    ym = ympool.tile([128, 512], BF16, tag="ym")
                    nc.vector.tensor_add(ym[:, 0:NC], aS[:, 0:NC], psB[:, 0:NC])
                    nc.vector.tensor_reduce(sscr[:, ci:ci + 1], ym[:, 0:NC],
                                            axis=mybir.AxisListType.X, op=ADD)
                    sq = y2pool.tile([128, 512], BF16, tag="sq")
                    nc.scalar.activation(sq[:, 0:NC], ym[:, 0:NC],
                                         mybir.ActivationFunctionType.Square,
                                         accum_out=s2scr[:, ci:ci + 1])

            # --- finalize batch ---
            # reduce scratch columns, fold bias per channel, then one
            # GpSimd cross-partition all-reduce:
            #   T1 = sum_c S + NPOS * sum_c b
            #   T2 = sum_c S2 + 2 * sum_c (b*S) + NPOS * sum_c b^2
            #   out = T1 / sqrt(NTOT*T2 - T1^2 + eps*NTOT^2)
            packed = accpool.tile([128, 8], F32, tag="packed")
            nc.vector.memset(packed[:, 3:8], 0.0)
            nc.vector.tensor_reduce(packed[:, 0:1], sscr[:, :],
                                    axis=mybir.AxisListType.X, op=ADD)
            nc.vector.tensor_reduce(packed[:, 1:2], s2scr[:, :],
                                    axis=mybir.AxisListType.X, op=ADD)
            # b*S on ScalarE (Identity with per-partition AP scale),
            # overlapping the S2 scratch reduce on DVE
            nc.scalar.activation(packed[:, 2:3], packed[:, 0:1],
                                 mybir.ActivationFunctionType.Identity,
                                 scale=bias_t[:, 0:1])
            red = accpool.tile([1, 3], F32, tag="red3")
            if b == 0:
                # hidden under batch-1's stream: SBUF->SBUF DMA flatten
                # + strided DVE reduce (keeps the PE queue untouched)
                cat = accpool.tile([1, 384], F32, tag="cat")
                nc.sync.dma_start(cat[0:1, 0:384], packed[:, 0:3])
                nc.vector.tensor_reduce(
                    red[0:1, 0:3],
                    cat[0:1, 0:384].rearrange("p (a b) -> p b a", b=3),
                    axis=mybir.AxisListType.X, op=ADD)
            else:
                # terminal: the PE is idle, so two stream-identical
                # row-tiled f32r matmuls against the ones-column matrix
                # sum the partition halves ~2us faster than the DMA hop
                pk = accpool.tile([128, 8], F32R, tag="pkr")
                nc.vector.tensor_copy(pk[:, 0:8], packed[:, 0:8])
                rpA = pspool.tile([128, 512], F32, tag="ps")
                rpB = pspool.tile([128, 512], F32, tag="ps")
                nc.tensor.matmul(rpA[:, 0:8], onesm[0:64, :], pk[0:64, 0:8],
                                 start=True, stop=True, tile_position=(0, 0))
                nc.tensor.matmul(rpB[:, 0:8], onesm[64:128, :], pk[64:128, 0:8],
                                 start=True, stop=True, tile_position=(64, 0))
                redA = accpool.tile([1, 3], F32, tag="redA")
                nc.scalar.copy(redA[0:1, 0:3], rpA[0:1, 0:3])
                nc.vector.tensor_add(red[0:1, 0:3], redA[0:1, 0:3],
                                     rpB[0:1, 0:3])

            f = finpool.tile([1, 8], F32, tag=f"fin{b}")
            # T1 = NPOS*cb0 + redS on ScalarE, overlapping the DVE
            # t2a/T2 chain (Identity = fused in*scale + bias)
            nc.scalar.activation(f[0:1, 0:1], cb[0:1, 0:1],
                                 mybir.ActivationFunctionType.Identity,
                                 bias=red[0:1, 0:1], scale=float(NPOS))
            # t2a = 2*red_bS + redS2 ; T2 = NPOS*cb1 + t2a
            nc.vector.scalar_tensor_tensor(
                f[0:1, 1:2], red[0:1, 2:3], 2.0, red[0:1, 1:2],
                op0=MULT, op1=ADD)
            nc.vector.scalar_tensor_tensor(
                f[0:1, 2:3], cb[0:1, 1:2], float(NPOS), f[0:1, 1:2],
                op0=MULT, op1=ADD)
            # m1 = T1*T1 - eps*NTOT^2 ; d = NTOT*T2 - m1
            nc.vector.scalar_tensor_tensor(
                f[0:1, 3:4], f[0:1, 0:1], f[0:1, 0:1], negeps_t[0:1, 0:1],
                op0=MULT, op1=ADD)
            nc.vector.scalar_tensor_tensor(
                f[0:1, 4:5], f[0:1, 2:3], float(NTOT), f[0:1, 3:4],
                op0=MULT, op1=SUB)
            # rsqrt in one ACT op (d > 0 always); table accuracy is far
            # inside the 2e-2 budget
            nc.scalar.activation(f[0:1, 5:6], f[0:1, 4:5],
                                 mybir.ActivationFunctionType.Abs_reciprocal_sqrt)
            nc.vector.tensor_mul(resb[0:1, b:b + 1], f[0:1, 0:1], f[0:1, 5:6])

        nc.sync.dma_start(out_ap[0:1, 0:NB], resb[0:1, 0:NB])


_NC_CACHE = None


def _module():
    global _NC_CACHE
    if _NC_CACHE is None:
        nc = bacc.Bacc("TRN2", target_bir_lowering=False, debug=False,
                       num_devices=N_CORES)
        _emit(nc)
        nc.compile()
        _NC_CACHE = nc
    return _NC_CACHE


def _prep_weights(conv_weight):
    wq = np.zeros((128, 2 * 14 * 128), dtype=np.float32)
    for s, (ta, tb) in enumerate(SPLITS):
        woff = s * 14 * 128
        for i, (kd, kh, kw) in enumerate(ta):
            wq[0:64, woff + i * 128:woff + (i + 1) * 128] = \
                conv_weight[:, :, kd, kh, kw].T
        for i, (kd, kh, kw) in enumerate(tb):
            wq[64:128, woff + i * 128:woff + (i + 1) * 128] = \
                conv_weight[:, :, kd, kh, kw].T
    return wq


def kernel(x, conv_weight, conv_bias):
    x = np.ascontiguousarray(np.asarray(x, dtype=np.float32))
    w = np.asarray(conv_weight, dtype=np.float32)
    bias = np.asarray(conv_bias, dtype=np.float32)

    wq = _prep_weights(w)
    bias2 = np.ascontiguousarray(bias.reshape(128, 1))
    xr = x.reshape(16, CIN, D * PL)

    in_maps = []
    for c in range(N_CORES):
        in_maps.append({
            "x": np.ascontiguousarray(xr[NB * c:NB * (c + 1)]),
            "wq": wq,
            "bias": bias2,
        })

    nc = _module()
    res = run_bass_kernel_spmd(nc, in_maps, core_ids=list(range(N_CORES)))

    out = np.empty(16, dtype=np.float32)
    for c in range(N_CORES):
        out[NB * c:NB * (c + 1)] = res.results[c]["out"].reshape(NB)
    return out
